# revision 16
# baseline (speedup 1.0000x reference)
"""TRN2 Bass kernel for the ConceptualMambaBlock problem (bf16 redesign).

Math (reference):
    x: [B=4, T=96, N=512, H=128] f32
    expanded = x @ W_exp.T + b_exp            # [B,T,N,2H]
    primary, gating = split(expanded, 2, -1)
    s_t = 0.9*s_{t-1} + 0.1*gating_t          # EMA along T
    out = (primary * sigmoid(s)) @ W_con.T + b_con

Restructure vs the fp32 baseline: the EMA is linear, so it commutes with
the gating Linear:

    s_t = (0.1*W_g) @ z_t + b_g*(1 - 0.9^t),   z_t = sum_{k<=t} 0.9^{t-k} x_k

  - The scan runs on the *input* x in SBUF at bf16 (4-block batched DVE
    ops) instead of on the matmul output in PSUM at fp32: both cheaper
    per element and independent of the matmul pipeline (pure prefetch).
  - The b_g*(1-0.9^t) term: +b_g rides the sigmoid's bias port; the
    -b_g*0.9^t part is injected through the scan via a "phantom column"
    per node: each node's 96 columns become 97, with column 0 holding
    the constant c = -10*Wg^-1*b_g (computed on host).  The scan reset
    lands on it (z_ph = c, z_1 = 0.9c + x_1, ...), so z carries an
    extra 0.9^t*c that maps to exactly -b_g*0.9^t through the gating
    matmul.  Both matmuls skip the phantom column via strided APs, so
    the primary path still sees raw x.  Zero extra instructions.
  - The 0.9^t weights inside the scan use a greedy per-column bf16
    multiplier sequence whose running products track 0.9^j to within
    one bf16 ulp (no compounding of the bf16 rounding of 0.9).

Everything runs in bf16 (PSUM and the scan state stay fp32 in HW):
PE matmuls at the bf16 rate, input+output HBM traffic halved.
Tolerance is 2e-2; measured error ~4e-3.

Pipeline: fine-grained 2-block iterations (one iteration of slack on
every cross-engine edge), mirroring the well-overlapped fp32 baseline:
  PE  g: mm1g(g) x2 -> pg | mm1p(g) x2 -> pp | mm2(g-1) x2 -> po
  ACT g: sigmoid(g) [pg -> bf16 gate] | Identity+b2 outcopy(g-1)
  DVE g: scan chunk (next group, no deps - emitted first so it fills
         any wait) | stt(g-1): y = (pp+b1p)*sig fused PSUM drain
PSUM banks (8 x [128,512] f32, one manually-managed tile):
  {0,1}/{2,3}: pg, iteration parity; freed by the sigmoid (2-iter reuse)
  {4,5}/{6,7}: pp, iteration parity; po(g-1) reuses the pair after the
               stt drain, freed by the outcopy.
The PE HAM starts throttled (K=4/8, 1.2 GHz): a prologue warm-up burst
plus one 8-matmul dummy burst after the pipeline is primed (iteration
2) push it to K=8/8; the steady-state PE gaps stay below the ~3.4us
re-throttle window, so it remains warm.
DMA: input via the sync HWDGE ring, output via the scalar HWDGE ring,
8-block groups (~790 KB).
"""

import numpy as np
import ml_dtypes

import concourse.bacc as bacc
import concourse.mybir as mybir
import concourse.tile as tile
from concourse.bass_utils import run_bass_kernel_spmd

F32 = mybir.dt.float32
BF16 = mybir.dt.bfloat16
AF = mybir.ActivationFunctionType
ALU = mybir.AluOpType

B, T, N, H = 4, 96, 512, 128
NCORES = 8
NLOC = N // 2          # 256 nodes per core
NB = 4                 # nodes per block
TP = T + 1             # 97 cols per node incl. phantom
TOK = NB * T           # 384 real columns per block
TOKX = NB * TP         # 388 stored columns per block
NBLK = NLOC // NB      # 64 blocks per core
SG = 8                 # blocks per DMA/scan group
NSG = NBLK // SG       # 8 groups
MG = 2                 # blocks per iteration
NMG = NBLK // MG       # 32 iterations
# scan runs as 1-block chunks (2 per iteration, kept 2 blocks ahead)
# so the long serial scan never delays the stt->mm2 chain on the DVE FIFO

_NC_CACHE = None


def _greedy_mask_pattern():
    """Per-node TP-column multiplier sequence: col 0 is 0.0 (reset, lands
    on the phantom c column); col j (j=1..96) is a bf16 value m_j chosen
    so prod(m_1..m_j) tracks 0.9^j to within one bf16 ulp."""
    ms = [0.0]
    c_act = 1.0
    tgt = 1.0
    for _ in range(T):
        tgt *= 0.9
        m = float(np.asarray(tgt / c_act, dtype=np.float32).astype(ml_dtypes.bfloat16))
        ms.append(m)
        c_act *= m
    return np.array(ms, dtype=np.float64)


def _build():
    nc = bacc.Bacc()

    xt_h = nc.dram_tensor("xt", [H, NBLK, NB, TP], BF16, kind="ExternalInput")
    wpack_h = nc.dram_tensor("wpack", [H, 3 * H], BF16, kind="ExternalInput")
    bias_h = nc.dram_tensor("bias", [H, 3], F32, kind="ExternalInput")
    mask_h = nc.dram_tensor("mask", [H, 2 * TOKX], BF16, kind="ExternalInput")
    out_h = nc.dram_tensor("out", [H, NBLK, TOK], BF16, kind="ExternalOutput")

    with tile.TileContext(nc) as tc:
        with (
            tc.tile_pool(name="consts", bufs=1) as cp,
            tc.tile_pool(name="io", bufs=1) as io,
            tc.tile_pool(name="mid", bufs=1) as mid,
            tc.tile_pool(name="ps", bufs=1, space="PSUM") as ps,
        ):
            state = {}
            xts = [None] * NSG
            zs = [None] * NSG
            obs = [None] * NSG

            def load_group(s):
                xg = io.tile([H, SG, NB, TP], BF16, tag="x", name=f"x{s}", bufs=3)
                nc.sync.dma_start(out=xg[:], in_=xt_h[:, s * SG : (s + 1) * SG, :, :])
                xts[s] = xg

            load_group(0)  # x(0) first on the sync ring

            wpack_sb = cp.tile([H, 3 * H], BF16, tag="wpack")
            nc.sync.dma_start(out=wpack_sb[:], in_=wpack_h[:, :])
            bias_sb = cp.tile([H, 3], F32, tag="bias")
            nc.sync.dma_start(out=bias_sb[:], in_=bias_h[:, :])
            mask_sb = cp.tile([H, 2 * TOKX], BF16, tag="mask")
            nc.sync.dma_start(out=mask_sb[:], in_=mask_h[:, :])

            load_group(1)

            w1pT = wpack_sb[:, 0:H]
            w1gT = wpack_sb[:, H : 2 * H]
            wcT = wpack_sb[:, 2 * H : 3 * H]
            bg_ap = bias_sb[:, 0:1]
            b1p_ap = bias_sb[:, 1:2]
            b2_ap = bias_sb[:, 2:3]

            psum = ps.tile([H, 8, 512], F32, tag="all")

            # prologue warm-up: fills the initial DMA/scan wait with PE
            # activity (bank 6's first real use is ~2 iterations in)
            for _ in range(10):
                nc.tensor.matmul(
                    psum[:, 6, 0:TOK], lhsT=wpack_sb[:, 0:H],
                    rhs=mask_sb[:, 0:TOK], start=True, stop=True,
                )

            def scan_pair(p):
                # one scan op covering blocks 2p, 2p+1 (FD = 2*TOKX)
                s0, h = (2 * p) // SG, (2 * p) % SG
                if zs[s0] is None:
                    zs[s0] = mid.tile([H, SG, NB, TP], BF16, tag="z", name=f"z{s0}", bufs=2)
                x2 = xts[s0][:, h : h + 2, :, :].rearrange("p a b c -> p (a b c)")
                z2 = zs[s0][:, h : h + 2, :, :].rearrange("p a b c -> p (a b c)")
                nc.vector.tensor_tensor_scan(
                    out=z2, data0=mask_sb[:], data1=x2,
                    initial=0.0, op0=ALU.mult, op1=ALU.add,
                )

            scan_pair(0)
            scan_pair(1)

            def emit_stt(g):
                # y = (pp + b1p) * sg, fused PSUM drain on DVE
                k1 = g % 2
                y_t = mid.tile([H, MG, TOK], BF16, tag="y", name=f"y{g % 4}", bufs=3)
                nc.vector.scalar_tensor_tensor(
                    out=y_t[:],
                    in0=psum[:, 4 + 2 * k1 : 6 + 2 * k1, 0:TOK],
                    scalar=b1p_ap,
                    in1=state[g]["sg"][:],
                    op0=ALU.add, op1=ALU.mult,
                )
                state[g]["y"] = y_t

            def emit_mm2_and_out(g):
                # po reuses pp's parity pair (already drained by the stt)
                k1 = g % 2
                s1 = g * MG // SG
                q1 = (g * MG % SG) // MG
                y_t = state[g]["y"]
                for j in range(MG):
                    nc.tensor.matmul(
                        psum[:, 4 + 2 * k1 + j, 0:TOK], lhsT=wcT, rhs=y_t[:, j, :],
                        start=True, stop=True,
                    )
                nc.scalar.activation(
                    obs[s1][:, q1 * MG : (q1 + 1) * MG, :],
                    psum[:, 4 + 2 * k1 : 6 + 2 * k1, 0:TOK],
                    AF.Identity, bias=b2_ap, scale=1.0,
                )
                if q1 == SG // MG - 1:
                    nc.scalar.dma_start(
                        out=out_h[:, s1 * SG : (s1 + 1) * SG, :], in_=obs[s1][:]
                    )
                del state[g]

            for g in range(NMG):
                s = g * MG // SG
                q = (g * MG % SG) // MG
                k = g % 2
                if q == 0:
                    if s + 2 < NSG:
                        load_group(s + 2)
                    obs[s] = io.tile([H, SG, TOK], BF16, tag="ob", name=f"ob{s}", bufs=2)

                # PE: gating matmuls
                for j in range(MG):
                    nc.tensor.matmul(
                        psum[:, 2 * k + j, 0:TOK], lhsT=w1gT,
                        rhs=zs[s][:, q * MG + j, :, 1:TP],
                        start=True, stop=True,
                    )

                # ACT: sigmoid as early as possible
                sg_t = mid.tile([H, MG, TOK], BF16, tag="sg", name=f"sg{g % 4}", bufs=3)
                nc.scalar.activation(
                    sg_t[:], psum[:, 2 * k : 2 * k + 2, 0:TOK],
                    AF.Sigmoid, bias=bg_ap, scale=1.0,
                )

                # PE: primary matmuls
                for j in range(MG):
                    nc.tensor.matmul(
                        psum[:, 4 + 2 * k + j, 0:TOK], lhsT=w1pT,
                        rhs=xts[s][:, q * MG + j, :, 1:TP],
                        start=True, stop=True,
                    )

                # one-time HAM warm burst once the pipeline is primed: 8
                # back-to-back dummies (>3.4us cold) flip the PE to 2.4 GHz;
                # steady-state gaps are short enough to keep it there.
                # pg bank 0's next writer is 2 iterations away - no stall.
                if g == 2:
                    for _ in range(8):
                        nc.tensor.matmul(
                            psum[:, 0, 0:TOK], lhsT=wpack_sb[:, 0:H],
                            rhs=mask_sb[:, 0:TOK], start=True, stop=True,
                        )

                # DVE: previous iteration's gate drain first (odd iters
                # route through ACT+GPSIMD instead), then one 2-block scan
                # chunk - keeps the scan 2 blocks ahead without ever
                # blocking the gate drain
                if g - 1 in state and "y" not in state[g - 1]:
                    emit_stt(g - 1)
                if g + 2 < NMG:
                    scan_pair(g + 2)

                state[g] = {"sg": sg_t}

                # odd iterations: gate via ACT drain + GPSIMD multiply,
                # freeing the DVE (the scarce engine) of half the stt work.
                # y(g) is consumed by mm2 in iteration g+1: the extra
                # ppb+gpsimd latency (~2.6us) fits inside that slack.
                if k == 1:
                    ppb = mid.tile([H, MG, TOK], BF16, tag="ppb", name="ppb", bufs=2)
                    nc.scalar.activation(
                        ppb[:], psum[:, 4 + 2 * k : 6 + 2 * k, 0:TOK],
                        AF.Identity, bias=b1p_ap, scale=1.0,
                    )
                    y_t = mid.tile([H, MG, TOK], BF16, tag="y", name=f"y{g % 4}", bufs=3)
                    nc.gpsimd.tensor_tensor(
                        out=y_t[:].rearrange("p a b -> p (a b)"),
                        in0=ppb[:].rearrange("p a b -> p (a b)"),
                        in1=sg_t[:].rearrange("p a b -> p (a b)"),
                        op=ALU.mult,
                    )
                    state[g]["y"] = y_t

                # PE: mm2 + ACT outcopy + DMA of g-1
                if g - 1 in state and "y" in state[g - 1]:
                    emit_mm2_and_out(g - 1)

            if "y" not in state[NMG - 1]:
                emit_stt(NMG - 1)
            emit_mm2_and_out(NMG - 1)

    nc.finalize()
    return nc


def _get_nc():
    global _NC_CACHE
    if _NC_CACHE is None:
        _NC_CACHE = _build()
    return _NC_CACHE


def _inj_vector(W_exp, b_exp):
    """c with (0.1*Wg) @ c == -b_g: the phantom-column payload."""
    Wg = W_exp[H:].astype(np.float64)
    bg = b_exp[H:].astype(np.float64)
    try:
        c = -10.0 * np.linalg.solve(Wg, bg)
        if not np.all(np.isfinite(c)) or np.abs(c).max() > 2000.0:
            raise np.linalg.LinAlgError
    except np.linalg.LinAlgError:
        c = -10.0 * np.linalg.lstsq(Wg, bg, rcond=1e-2)[0]
    return c


def _in_maps(x, W_exp, b_exp, W_con, b_con):
    bf16 = ml_dtypes.bfloat16
    wpack = np.concatenate(
        [W_exp[:H, :].T, (0.1 * W_exp[H:, :]).T, W_con.T], axis=1
    ).astype(bf16)
    wpack = np.ascontiguousarray(wpack)

    bias = np.stack([b_exp[H:], b_exp[:H], b_con], axis=1).astype(np.float32)
    bias = np.ascontiguousarray(bias)

    mpat = _greedy_mask_pattern()                    # [97]
    mask = np.tile(mpat, 2 * NB)[None, :].repeat(H, axis=0).astype(bf16)
    mask = np.ascontiguousarray(mask)

    c = _inj_vector(W_exp, b_exp)

    maps = []
    for c_id in range(NCORES):
        bb, nh = c_id // 2, c_id % 2
        xs = x[bb, :, nh * NLOC : (nh + 1) * NLOC, :]  # [T, NLOC, H]
        xT = xs.transpose(2, 1, 0)                     # [H, NLOC, T]
        xhat = np.empty((H, NLOC, TP), dtype=np.float64)
        xhat[:, :, 0] = c[:, None]
        xhat[:, :, 1:] = xT
        maps.append(
            {
                "xt": np.ascontiguousarray(xhat.astype(bf16)).reshape(H, NBLK, NB, TP),
                "wpack": wpack,
                "bias": bias,
                "mask": mask,
            }
        )
    return maps


def run_spmd(x, W_exp, b_exp, W_con, b_con, **spmd_kwargs):
    """Run the 8-core kernel; returns (full_output, BassKernelResults)."""
    maps = _in_maps(x, W_exp, b_exp, W_con, b_con)
    res = run_bass_kernel_spmd(
        _get_nc(), maps, core_ids=list(range(NCORES)), **spmd_kwargs
    )
    out = np.empty((B, T, N, H), dtype=np.float32)
    for c_id in range(NCORES):
        bb, nh = c_id // 2, c_id % 2
        oT = res.results[c_id]["out"].astype(np.float32).reshape(H, NLOC, T)
        out[bb, :, nh * NLOC : (nh + 1) * NLOC, :] = oT.transpose(2, 1, 0)
    return out, res


def kernel(spatial_temporal_representation, W_exp, b_exp, W_con, b_con):
    out, _ = run_spmd(
        np.asarray(spatial_temporal_representation, dtype=np.float32),
        np.asarray(W_exp, dtype=np.float32),
        np.asarray(b_exp, dtype=np.float32),
        np.asarray(W_con, dtype=np.float32),
        np.asarray(b_con, dtype=np.float32),
    )
    return out
